# revision 28
# baseline (speedup 1.0000x reference)
"""Linear multi-head attention (ELU+1 feature map) Trainium2 Bass kernel, v3.

Full inputs in, full output out. Sharding: 8 cores = (batch b, seq-half h);
core i handles batch i//2, sequence columns [h*2048, (h+1)*2048).

Math: out = wo @ BD(ctx^T) @ wq @ x + bias terms, where
  ctx[h] = k'[h] @ v[h]^T summed over the full sequence (AllGather over the
  2-core pair), BD = block-diagonal of the per-head ctx^T blocks.
So instead of q-proj / attn / out-proj we form M^T = (BD @ wq)^T-contracted
with wo once per core and apply out = M @ x (one projection).

All matmul inputs are bf16 (host-cast); accumulation fp32 in PSUM.

v3 over v2:
- x / wk / wv are host-pretiled into partition-major layouts so every DMA
  is a 2KB-contiguous run per partition, and phase 1 is restructured into
  (stage, si, seg) work units of one o-quarter (2 head-pairs, 256 cols)
  each: only 512KB (wk q0 + x si0) gates the first matmul, quarter-2 units
  for si 0-3 are backfilled during si 4-7, and weights stream just-in-time
  across the SP/Act/Pool queues.
- The per-seg ctx psums accumulate in any si order (sum over s), one PSUM
  bank per seg -- interleaved accumulation groups must NOT share a bank
  (per-bank accumulation state on the PE corrupts results; measured) --
  with the previous unit's ctx matmuls pipelined into the next unit.
- The stage-A AllGather keeps ~27us of stage-B work as skew cover, and its
  bd assembly stays hoisted into the stage-B DVE stream.
- (Evaluated and rejected: fp8 DoubleRow matmuls -- rel err 3.7e-2 exceeds
  the 2e-2 gate; pair-deduplicated M formation via a 1MB AllReduce -- the
  CC fabric moves ~35GB/s so the exchange costs ~30us against 13.7us of PE
  saved and stalls the PE ~20us; PE p-state warmup matmuls -- counter-
  productive under ambient down-clock windows.)
"""

import numpy as np
import ml_dtypes

import jax
from jax.sharding import Mesh, NamedSharding, PartitionSpec

from concourse import bass, bacc, tile, mybir
from concourse.bass2jax import (
    _bass_exec_p,
    install_neuronx_cc_hook,
    partition_id_tensor,
)

from jax.experimental.shard_map import shard_map

F32 = mybir.dt.float32
BF16 = mybir.dt.bfloat16
ACT = mybir.ActivationFunctionType
BF16_NP = ml_dtypes.bfloat16

N_CORES = 8
B, C, S = 4, 1024, 4096
H, DH = 16, 64
S_LOC = S // 2          # per-core sequence columns
NCH = C // 128          # contraction chunks (8)
PAIRS = C // 128        # head pairs = 8 (each pair = 128 channels)
N_SI = S_LOC // 128     # s-chunks per half-loop (16)
N_SB = S_LOC // 512     # out s-blocks (4)
REPLICAS = [[0, 1], [2, 3], [4, 5], [6, 7]]
# phase-1 stages as (first_pair, n_pairs, segments): a 6/2 split so the
# first AllGather covers 6 of 8 head-pairs and the trailing collective
# (2 pairs) hides under the first stage's T1/M^T work. Stage A computes
# its 768 output channels per s-chunk as a 512-wide + 256-wide segment
# to stay within PSUM bank-sized psum tiles.
STAGES6 = [
    (0, 6, [(0, 4), (4, 2)]),
    (6, 2, [(6, 2)]),
]


def build_program(has_kv_bias=False, has_q_bias=False):
    nc = bacc.Bacc(
        "TRN2", target_bir_lowering=False, debug=False, num_devices=N_CORES
    )

    # x is host-pretiled to [128, si, ci, 128] so every per-si DMA is a
    # 2KB-contiguous run per partition (vs 512B with the flat [C, S] layout)
    x_d = nc.dram_tensor("x", [128, N_SI, NCH, 128], BF16, kind="ExternalInput")
    # wk/wv are host-pretiled to [128, o-quarter, ci, 256] so the first
    # (PE-gating) weight tranches are 2KB-contiguous per partition
    wkt_d = nc.dram_tensor("wkt", [128, 4, NCH, 256], BF16, kind="ExternalInput")
    wvt_d = nc.dram_tensor("wvt", [128, 4, NCH, 256], BF16, kind="ExternalInput")
    wqn_d = nc.dram_tensor("wqn", [C, C], BF16, kind="ExternalInput")
    wot_d = nc.dram_tensor("wot", [C, C], BF16, kind="ExternalInput")
    bo_col_d = nc.dram_tensor("bo_col", [128, NCH], F32, kind="ExternalInput")
    bk_d = nc.dram_tensor("bk_row", [1, C], BF16, kind="ExternalInput")
    bv_d = nc.dram_tensor("bv_row", [1, C], BF16, kind="ExternalInput")
    bq_d = nc.dram_tensor("bq_col", [128, NCH], BF16, kind="ExternalInput")
    ones_d = nc.dram_tensor("ones", [1, 128], BF16, kind="ExternalInput")
    out_d = nc.dram_tensor("out", [C, S_LOC], F32, kind="ExternalOutput")

    with tile.TileContext(nc) as tc:
        with (
            tc.tile_pool(name="const", bufs=1) as const,
            # kv/tmp stay open for the whole program: closing them mid-run
            # would let phase-2 SBUF tiles reuse their region, serializing
            # the first T1 writes behind the phase-1 drain
            tc.tile_pool(name="kv", bufs=4) as kv,
            tc.tile_pool(name="tmp", bufs=5) as tmp,
            # one program-wide PSUM ring for every matmul output: pool
            # transitions between phases otherwise serialize the next
            # phase's first matmuls behind the previous pool's drain
            tc.tile_pool(name="ps_all", bufs=5, space="PSUM") as ps_all,
            tc.tile_pool(name="ps_ctx", bufs=3, space="PSUM") as ps_ctx,
            tc.tile_pool(name="dram", bufs=1, space="DRAM") as dram,
        ):
            # ---- resident tensors; DMA order = sync queue order ----------
            wk_sb = const.tile([128, 4, NCH, 256], BF16, tag="wk")
            wv_sb = const.tile([128, 4, NCH, 256], BF16, tag="wv")
            x_sb = const.tile([128, N_SI, NCH, 128], BF16, tag="x")
            wk_ap = wkt_d.ap()
            wv_ap = wvt_d.ap()
            x_ap = x_d.ap()

            def wslice(w_sb, ci, ho, w):
                # rhs AP for o-columns [ho, ho+w) of chunk ci in the
                # o-quarter-major layout; ho/w are multiples of 256
                q0 = ho // 256
                return w_sb[:, q0 : q0 + w // 256, ci, :]

            # Two-queue (SP hwdge + Pool swdge) schedule in first-use order.
            # Stage-A's narrow seg (o 512:768) is deferred+backfilled in the
            # unit loop below, so only seg0 weights + x si0 gate the PE.
            # The hwdge queues round-robin their in-flight entries (an
            # entry completes after ~size/(bw/n_coresident)), so keep them
            # nearly empty at the start: the PE-gating pieces ride hwdge
            # alone, and the bulk rides the Pool swdge queue's separate
            # rings in deadline order.
            # sync+scalar share the hwdge rings, and entries round-robin:
            # ONLY the first-matmul gate (x si0 + wk q0, 768KB) rides hwdge
            # so it completes at gate_bytes/bw; everything else rides the
            # Pool swdge queue's separate rings, trigger-ordered by deadline
            nc.sync.dma_start(x_sb[:, 0:1], x_ap[:, 0:1])
            nc.sync.dma_start(wk_sb[:, 0:1, 0:4], wk_ap[:, 0:1, 0:4])
            nc.scalar.dma_start(wk_sb[:, 0:1, 4:8], wk_ap[:, 0:1, 4:8])
            nc.gpsimd.dma_start(wv_sb[:, 0:1], wv_ap[:, 0:1])
            nc.gpsimd.dma_start(x_sb[:, 1:2], x_ap[:, 1:2])
            nc.gpsimd.dma_start(wk_sb[:, 1:2], wk_ap[:, 1:2])
            nc.gpsimd.dma_start(wv_sb[:, 1:2], wv_ap[:, 1:2])
            bias_tiles = {}
            if has_kv_bias:
                bk_row = const.tile([1, C], BF16, tag="bk")
                nc.sync.dma_start(bk_row[:], bk_d[:])
                bv_row = const.tile([1, C], BF16, tag="bv")
                nc.sync.dma_start(bv_row[:], bv_d[:])
                ones_row = const.tile([1, 128], BF16, tag="ones")
                nc.sync.dma_start(ones_row[:], ones_d[:])
                bias_tiles.update(bk=bk_row, bv=bv_row, ones=ones_row)
            nc.gpsimd.dma_start(x_sb[:, 2:4], x_ap[:, 2:4])
            nc.gpsimd.dma_start(x_sb[:, 4:6], x_ap[:, 4:6])
            nc.gpsimd.dma_start(wk_sb[:, 2:3], wk_ap[:, 2:3])
            nc.gpsimd.dma_start(wv_sb[:, 2:3], wv_ap[:, 2:3])
            nc.gpsimd.dma_start(x_sb[:, 6:8], x_ap[:, 6:8])
            nc.gpsimd.dma_start(x_sb[:, 8:10], x_ap[:, 8:10])
            nc.gpsimd.dma_start(x_sb[:, 10:12], x_ap[:, 10:12])
            nc.gpsimd.dma_start(x_sb[:, 12:14], x_ap[:, 12:14])
            nc.gpsimd.dma_start(x_sb[:, 14:16], x_ap[:, 14:16])
            nc.gpsimd.dma_start(wk_sb[:, 3:4], wk_ap[:, 3:4])
            nc.gpsimd.dma_start(wv_sb[:, 3:4], wv_ap[:, 3:4])

            wqn_sb = const.tile([128, NCH, C], BF16, tag="wqn")
            nc.sync.dma_start(
                wqn_sb[:], wqn_d.ap().rearrange("(n p) c -> p n c", p=128)
            )
            wot_sb = const.tile([128, NCH, C], BF16, tag="wot")
            nc.gpsimd.dma_start(
                wot_sb[:], wot_d.ap().rearrange("(n p) o -> p n o", p=128)
            )
            bo_col = const.tile([128, NCH], F32, tag="bo")
            nc.sync.dma_start(bo_col[:], bo_col_d[:])
            if has_q_bias:
                bq_col = const.tile([128, NCH], BF16, tag="bq")
                nc.sync.dma_start(bq_col[:], bq_d[:])

            # block-diagonal ctx^T lhsT tiles: memset-zeroed up front, diagonal
            # 64x64 blocks overwritten after each AllGather lands.
            bd, raw_ab, ar_in, ar_out = [], [], [], []
            for h, (fp, npr, _segs) in enumerate(STAGES6):
                w = npr * 128
                bd_h = const.tile([128, w], BF16, tag=f"bd{h}", name=f"bd{h}")
                nc.gpsimd.memset(bd_h[:], 0.0)
                bd.append(bd_h)
                raw_ab.append(
                    [
                        const.tile(
                            [128, w], BF16, tag=f"raw{h}{g}", name=f"raw{h}{g}"
                        )
                        for g in range(2)
                    ]
                )
                ar_in.append(
                    dram.tile([128, w], BF16, tag=f"ari{h}", name=f"ari{h}")
                )
                ar_out.append(
                    dram.tile([2, 128, w], BF16, tag=f"aro{h}", name=f"aro{h}")
                )

            # ---------------- Phase 1: k/v projections + context ----------
            # Work units (stage, si, seg), one o-quarter (2 head-pairs, 256
            # cols) each, so the first matmul is gated on just 512KB (wk q0
            # + x si0) and each quarter's weights stream in just-in-time.
            # Quarter 2 is deferred for si 0-3 and backfilled during si 4-7.
            # ctx accumulates per seg in any si order (it's a sum over s);
            # each unit's ctx matmuls are emitted one unit later (software
            # pipelining keeps the PE stream dense while the feature map is
            # in flight). Segs 0+1 share one psum bank (col regions 0:256 /
            # 256:512).
            SEGS = [(0, 2), (2, 2), (4, 2), (6, 2)]  # seg -> (first_pair, n)
            # one PSUM bank per seg: interleaved accumulation groups must
            # not share a bank (per-bank accumulation state on the PE)
            SEG_TILE = [(0, 0), (1, 0), (2, 0), (3, 0)]  # seg -> (bank key, col off)
            units = []
            for si in range(N_SI):
                units.append((0, si, 0))
                units.append((0, si, 1))
                if 4 <= si < 8:
                    units.append((0, si - 4, 2))
                if si >= 4:
                    units.append((0, si, 2))
            for si in range(N_SI):
                units.append((1, si, 3))

            def ctx_dst(sg):
                key, roff = SEG_TILE[sg]
                if key not in ctx_tiles:
                    ctx_tiles[key] = ps_ctx.tile(
                        [128, 512], F32, tag="ctx", name=f"ctx{key}"
                    )  # bank-sized; only [0:256] used
                return ctx_tiles[key], roff

            def emit_stage_gather(st):
                # partial context -> SBUF -> DRAM, AllGather the pair.
                # GPSIMD cannot read PSUM so the evict runs on DVE; the
                # DRAM staging DMA + collective go on the Pool queue, and
                # the gathered results are pulled back on the sync queue.
                fp, npr, _ = STAGES6[st]
                w_st = npr * 128
                ctx_sb = tmp.tile([128, 768], BF16, tag=f"ctxsb{st}")
                if st == 0:
                    for g in range(3):
                        nc.vector.tensor_copy(
                            ctx_sb[:, g * 256 : g * 256 + 256],
                            ctx_tiles[g][:, 0:256],
                        )
                else:
                    nc.vector.tensor_copy(
                        ctx_sb[:, 0:256], ctx_tiles[3][:, 0:256]
                    )
                nc.gpsimd.dma_start(ar_in[st][:], ctx_sb[:, 0:w_st])
                nc.gpsimd.collective_compute(
                    "AllGather",
                    mybir.AluOpType.bypass,
                    replica_groups=REPLICAS,
                    ins=[ar_in[st].opt()],
                    outs=[ar_out[st].opt()],
                )
                nc.sync.dma_start(raw_ab[st][0][:], ar_out[st][0:1, :, :])
                nc.sync.dma_start(raw_ab[st][1][:], ar_out[st][1:2, :, :])

            ctx_tiles = {}
            seg_done = [0, 0, 0, 0]
            prev = None  # (seg, kp_t, vt_t) of the previous unit

            def emit_ctx_of_prev():
                psg, pkp, pvt = prev
                pnp = SEGS[psg][1]
                dst, roff = ctx_dst(psg)
                seg_done[psg] += 1
                for pl in range(pnp):
                    po = roff + pl * 128
                    nc.tensor.matmul(
                        dst[:, po : po + 128],
                        pkp[:, pl * 128 : pl * 128 + 128],
                        pvt[:, pl * 128 : pl * 128 + 128],
                        start=(seg_done[psg] == 1 and pl == 0),
                        stop=(seg_done[psg] == N_SI and pl == pnp - 1),
                        skip_group_check=True,
                    )
                if (
                    psg in (0, 1, 2)
                    and seg_done[0] == N_SI
                    and seg_done[1] == N_SI
                    and seg_done[2] == N_SI
                ):
                    seg_done[psg] += 1  # fire the stage-A gather exactly once
                    emit_stage_gather(0)

            for st, si, sg in units:
                sfp, snp = SEGS[sg]
                ho = sfp * 128
                w = snp * 128
                # k^T chunk [s=128, o=w]
                pk = ps_all.tile([128, 512], F32, tag="pp")
                if has_kv_bias:
                    nc.tensor.matmul(
                        pk[:, 0:w],
                        bias_tiles["ones"][:],
                        bias_tiles["bk"][:, ho : ho + w],
                        start=True,
                        stop=False,
                    )
                for ci in range(NCH):
                    nc.tensor.matmul(
                        pk[:, 0:w],
                        x_sb[:, si, ci, :],
                        wslice(wk_sb, ci, ho, w),
                        start=(ci == 0 and not has_kv_bias),
                        stop=(ci == NCH - 1),
                    )
                # feature map: k' = exp(min(k,0)) + relu(k); min (DVE) and
                # relu (Scalar) read pk in parallel, so the serial chain is
                # 3 deep (min -> exp -> add) instead of 4
                kp_t = kv.tile([128, 512], BF16, tag="kp")
                r_t = tmp.tile([128, 512], F32, tag="r")
                nc.scalar.activation(r_t[:, 0:w], pk[:, 0:w], ACT.Relu)
                m_t = tmp.tile([128, 512], F32, tag="m")
                nc.vector.tensor_scalar(
                    m_t[:, 0:w], pk[:, 0:w], 0.0, None, mybir.AluOpType.min
                )
                nc.scalar.activation(m_t[:, 0:w], m_t[:, 0:w], ACT.Exp)
                nc.vector.tensor_add(kp_t[:, 0:w], m_t[:, 0:w], r_t[:, 0:w])

                # v^T chunk [s=128, o=w]
                pv = ps_all.tile([128, 512], F32, tag="pp")
                if has_kv_bias:
                    nc.tensor.matmul(
                        pv[:, 0:w],
                        bias_tiles["ones"][:],
                        bias_tiles["bv"][:, ho : ho + w],
                        start=True,
                        stop=False,
                    )
                for ci in range(NCH):
                    nc.tensor.matmul(
                        pv[:, 0:w],
                        x_sb[:, si, ci, :],
                        wslice(wv_sb, ci, ho, w),
                        start=(ci == 0 and not has_kv_bias),
                        stop=(ci == NCH - 1),
                    )
                vt_t = kv.tile([128, 512], BF16, tag="vt")
                nc.scalar.activation(vt_t[:, 0:w], pv[:, 0:w], ACT.Copy)

                if prev is not None:
                    emit_ctx_of_prev()
                prev = (sg, kp_t, vt_t)

                # hoist stage A's bd-adds into stage B's DVE stream
                # (its AllGather has landed by now) so the T1 matmuls
                # can start the moment the PE drains phase 1
                if st == 1 and si == 14:
                    fp0, npr0, _s0 = STAGES6[0]
                    raw_a0, raw_b0 = raw_ab[0]
                    for pl in range(npr0):
                        po = pl * 128
                        nc.vector.tensor_add(
                            bd[0][0:64, po : po + 64],
                            raw_a0[0:64, po : po + 64],
                            raw_b0[0:64, po : po + 64],
                        )
                        nc.vector.tensor_add(
                            bd[0][64:128, po + 64 : po + 128],
                            raw_a0[64:128, po + 64 : po + 128],
                            raw_b0[64:128, po + 64 : po + 128],
                        )

            emit_ctx_of_prev()  # flush the final stage-B unit
            emit_stage_gather(1)

            # ---------------- Phase 2: M^T = (wo @ BD @ wq)^T, out = M x --
            with (
                tc.tile_pool(name="p2", bufs=1) as p2,
                tc.tile_pool(name="p2t", bufs=3) as p2t,
            ):
                t1_sb = p2.tile([128, NCH, C], BF16, tag="t1")
                m0_sb = p2.tile([128, NCH, C], BF16, tag="m0")
                mt_sb = p2.tile([128, NCH, C], BF16, tag="mt")
                c1_sb = p2.tile([128, NCH], BF16, tag="c1") if has_q_bias else None

                for stage, (fp, npr, _segs) in enumerate(STAGES6):
                    # sum the gathered pair partials on the diagonal 64x64
                    # blocks only, straight into the zeroed bd tiles
                    # (stage 0's adds were hoisted into the phase-1 stream)
                    raw_a, raw_b = raw_ab[stage]
                    bd_h = bd[stage]
                    if stage > 0:
                        for pl in range(npr):
                            po = pl * 128
                            nc.vector.tensor_add(
                                bd_h[0:64, po : po + 64],
                                raw_a[0:64, po : po + 64],
                                raw_b[0:64, po : po + 64],
                            )
                            nc.vector.tensor_add(
                                bd_h[64:128, po + 64 : po + 128],
                                raw_a[64:128, po + 64 : po + 128],
                                raw_b[64:128, po + 64 : po + 128],
                            )

                    # T1 rows for this stage's pairs: T1[pp] = bd[pp].T @ wq[pp rows]
                    # chalf-major order matches the M loop's consumption
                    # (cc 0-3 read chalf 0), and the PSUM evictions alternate
                    # Scalar/DVE so they keep pace with the M matmul stream
                    for chalf in range(2):
                        co = chalf * 512
                        for pl in range(npr):
                            pp = fp + pl
                            # ps_ctx sits idle in phase 2; using it for the
                            # T1 psums decouples them from the M ring, and
                            # half-split evictions across Scalar+DVE turn
                            # the ring at the T1 matmul pace
                            t1_ps = ps_ctx.tile([128, 512], F32, tag="ctx")
                            nc.tensor.matmul(
                                t1_ps[:],
                                bd_h[:, pl * 128 : pl * 128 + 128],
                                wqn_sb[:, pp, co : co + 512],
                            )
                            nc.scalar.activation(
                                t1_sb[:, pp, co : co + 256],
                                t1_ps[:, 0:256],
                                ACT.Copy,
                            )
                            nc.vector.tensor_copy(
                                t1_sb[:, pp, co + 256 : co + 512],
                                t1_ps[:, 256:512],
                            )

                    # M^T partial over this stage's r-chunks:
                    # stage 0 -> m0_sb (bf16 staging), stage 1 -> += -> mt_sb
                    for cc in range(NCH):
                        for ohalf in range(2):
                            oo = ohalf * 512
                            m_ps = ps_all.tile([128, 512], F32, tag="pp")
                            for rl in range(npr):
                                rr = fp + rl
                                nc.tensor.matmul(
                                    m_ps[:],
                                    t1_sb[:, rr, cc * 128 : cc * 128 + 128],
                                    wot_sb[:, rr, oo : oo + 512],
                                    start=(rl == 0),
                                    stop=(rl == npr - 1),
                                )
                            if stage == 0:
                                nc.vector.tensor_copy(
                                    m0_sb[:, cc, oo : oo + 512], m_ps[:]
                                )
                            else:
                                nc.vector.tensor_add(
                                    mt_sb[:, cc, oo : oo + 512],
                                    m_ps[:],
                                    m0_sb[:, cc, oo : oo + 512],
                                )

                # output bias column: obias = wo @ (BD @ bq) + bo
                if has_q_bias:
                    for stage, (fp, npr, _segs) in enumerate(STAGES6):
                        for pl in range(npr):
                            pp = fp + pl
                            c1_ps = ps_all.tile([128, 512], F32, tag="pp")
                            nc.tensor.matmul(
                                c1_ps[:, 0:1],
                                bd[stage][:, pl * 128 : pl * 128 + 128],
                                bq_col[:, pp : pp + 1],
                            )
                            nc.vector.tensor_copy(c1_sb[:, pp : pp + 1], c1_ps[:, 0:1])
                    obias_col = p2.tile([128, NCH], F32, tag="obias")
                    for oc in range(NCH):
                        ob_ps = ps_all.tile([128, 512], F32, tag="pp")
                        for rc in range(NCH):
                            nc.tensor.matmul(
                                ob_ps[:, 0:1],
                                wot_sb[:, rc, oc * 128 : oc * 128 + 128],
                                c1_sb[:, rc : rc + 1],
                                start=(rc == 0),
                                stop=(rc == NCH - 1),
                            )
                        nc.vector.tensor_add(
                            obias_col[:, oc : oc + 1], ob_ps[:, 0:1], bo_col[:, oc : oc + 1]
                        )
                else:
                    obias_col = bo_col

                # ---- out = M @ x (+obias) ----
                for sb in range(N_SB):
                    ss = sb * 512
                    for oc in range(NCH):
                        po_t = ps_all.tile([128, 512], F32, tag="pp")
                        for ci in range(NCH):
                            nc.tensor.matmul(
                                po_t[:],
                                mt_sb[:, ci, oc * 128 : oc * 128 + 128],
                                x_sb[:, sb * 4 : sb * 4 + 4, ci, :],
                                start=(ci == 0),
                                stop=(ci == NCH - 1),
                            )
                        o_t = p2t.tile([128, 512], F32, tag="o")
                        if sb == N_SB - 1 and oc == NCH - 1:
                            # split the final tile so its eviction/DMA tail
                            # pipelines across two queues
                            for jo, q in ((0, nc.gpsimd), (256, nc.sync)):
                                nc.scalar.activation(
                                    o_t[:, jo : jo + 256],
                                    po_t[:, jo : jo + 256],
                                    ACT.Identity,
                                    bias=obias_col[:, oc : oc + 1],
                                )
                                q.dma_start(
                                    out_d[
                                        oc * 128 : oc * 128 + 128,
                                        ss + jo : ss + jo + 256,
                                    ],
                                    o_t[:, jo : jo + 256],
                                )
                        else:
                            nc.scalar.activation(
                                o_t[:],
                                po_t[:],
                                ACT.Identity,
                                bias=obias_col[:, oc : oc + 1],
                            )
                            nc.gpsimd.dma_start(
                                out_d[oc * 128 : oc * 128 + 128, ss : ss + 512],
                                o_t[:],
                            )


    nc.compile()
    return nc


# ---------------------------------------------------------------------------
# Host-side runner: mirrors run_bass_via_pjrt's multi-core path but caches the
# jitted callable (no donation) so repeat calls don't retrace.
# ---------------------------------------------------------------------------

_CACHE = {}


def _build_runner(key=(False, False)):
    if key in _CACHE:
        return _CACHE[key]

    install_neuronx_cc_hook()
    nc = build_program(*key)

    partition_name = nc.partition_id_tensor.name if nc.partition_id_tensor else None
    in_names, out_names, out_avals = [], [], []
    for alloc in nc.m.functions[0].allocations:
        if not isinstance(alloc, mybir.MemoryLocationSet):
            continue
        name = alloc.memorylocations[0].name
        if alloc.kind == "ExternalInput":
            if name != partition_name:
                in_names.append(name)
        elif alloc.kind == "ExternalOutput":
            out_names.append(name)
            out_avals.append(
                jax.core.ShapedArray(
                    tuple(alloc.tensor_shape), mybir.dt.np(alloc.dtype)
                )
            )
    n_params = len(in_names)
    all_in_names = list(in_names) + list(out_names)
    if partition_name is not None:
        all_in_names.append(partition_name)

    def _body(*args):
        operands = list(args)
        if partition_name is not None:
            operands.append(partition_id_tensor())
        outs = _bass_exec_p.bind(
            *operands,
            out_avals=tuple(out_avals),
            in_names=tuple(all_in_names),
            out_names=tuple(out_names),
            lowering_input_output_aliases=(),
            sim_require_finite=True,
            sim_require_nnan=True,
            nc=nc,
        )
        return tuple(outs)

    devices = jax.devices()[:N_CORES]
    mesh = Mesh(np.asarray(devices), ("core",))
    n_outs = len(out_names)
    fn = jax.jit(
        shard_map(
            _body,
            mesh=mesh,
            in_specs=(PartitionSpec("core"),) * (n_params + n_outs),
            out_specs=(PartitionSpec("core"),) * n_outs,
            check_rep=False,
        ),
        keep_unused=True,
    )
    sharding = NamedSharding(mesh, PartitionSpec("core"))
    runner = dict(
        fn=fn,
        in_names=in_names,
        out_names=out_names,
        out_avals=out_avals,
        sharding=sharding,
    )
    _CACHE[key] = runner
    return runner


def _pack_inputs(runner, in_maps):
    concat = [
        np.concatenate([np.asarray(m[name]) for m in in_maps], axis=0)
        for name in runner["in_names"]
    ]
    zeros = [
        np.zeros((N_CORES * a.shape[0], *a.shape[1:]), a.dtype)
        for a in runner["out_avals"]
    ]
    sh = runner["sharding"]
    return [jax.device_put(c, sh) for c in concat] + [
        jax.device_put(z, sh) for z in zeros
    ]


def _run(runner, in_maps):
    args = _pack_inputs(runner, in_maps)
    outs = runner["fn"](*args)
    results = []
    for ci in range(N_CORES):
        r = {}
        for i, name in enumerate(runner["out_names"]):
            full = np.asarray(outs[i])
            per = full.reshape(N_CORES, *runner["out_avals"][i].shape)
            r[name] = per[ci]
        results.append(r)
    return results


def make_in_maps(x, wq, bq, wk, bk, wv, bv, wo, bo):
    x = np.asarray(x, np.float32)
    def pret_w(w):
        # w^T [c, o] -> [128p, o-quarter, ci, 256]
        wt = np.asarray(w, np.float32).T
        return np.ascontiguousarray(
            wt.reshape(NCH, 128, 4, 256).transpose(1, 2, 0, 3)
        ).astype(BF16_NP)

    wktb = pret_w(wk)
    wvtb = pret_w(wv)
    wqnb = np.ascontiguousarray(np.asarray(wq, np.float32)).astype(BF16_NP)
    wotb = np.ascontiguousarray(np.asarray(wo, np.float32).T).astype(BF16_NP)
    bqa = np.asarray(bq, np.float32)
    bka = np.asarray(bk, np.float32)
    bva = np.asarray(bv, np.float32)
    boa = np.asarray(bo, np.float32)
    bo_col = np.ascontiguousarray(boa.reshape(NCH, 128).T)
    bq_col = np.ascontiguousarray(bqa.reshape(NCH, 128).T).astype(BF16_NP)
    bk_row = bka.reshape(1, C).astype(BF16_NP)
    bv_row = bva.reshape(1, C).astype(BF16_NP)
    ones = np.ones((1, 128), BF16_NP)
    in_maps = []
    for i in range(N_CORES):
        b, hh = i // 2, i % 2
        xi = x[b, :, hh * S_LOC : (hh + 1) * S_LOC]
        # pretile to [128p, si, ci, 128]: per-si DMAs are 2KB-contiguous
        xi = np.ascontiguousarray(
            xi.reshape(NCH, 128, N_SI, 128).transpose(1, 2, 0, 3)
        ).astype(BF16_NP)
        in_maps.append(
            dict(
                x=xi, wkt=wktb, wvt=wvtb, wqn=wqnb, wot=wotb,
                bo_col=bo_col, bq_col=bq_col, bk_row=bk_row, bv_row=bv_row,
                ones=ones,
            )
        )
    return in_maps


def kernel(x, wq, bq, wk, bk, wv, bv, wo, bo, num_heads):
    assert int(num_heads) == H
    x = np.asarray(x, np.float32)
    assert x.shape == (B, C, S), x.shape

    has_kv_bias = bool(np.any(np.asarray(bk)) or np.any(np.asarray(bv)))
    has_q_bias = bool(np.any(np.asarray(bq)))
    runner = _build_runner((has_kv_bias, has_q_bias))
    in_maps = make_in_maps(x, wq, bq, wk, bk, wv, bv, wo, bo)
    results = _run(runner, in_maps)

    out = np.empty((B, C, S), np.float32)
    for i in range(N_CORES):
        b, hh = i // 2, i % 2
        out[b, :, hh * S_LOC : (hh + 1) * S_LOC] = results[i]["out"]
    return out



# revision 30
# speedup vs baseline: 1.3866x; 1.3866x over previous
"""Linear multi-head attention (ELU+1 feature map) Trainium2 Bass kernel, v3.

Full inputs in, full output out. Sharding: 8 cores = (batch b, seq-half h);
core i handles batch i//2, sequence columns [h*2048, (h+1)*2048).

Math: out = wo @ BD(ctx^T) @ wq @ x + bias terms, where
  ctx[h] = k'[h] @ v[h]^T summed over the full sequence (AllGather over the
  2-core pair), BD = block-diagonal of the per-head ctx^T blocks.
So instead of q-proj / attn / out-proj we form M^T = (BD @ wq)^T-contracted
with wo once per core and apply out = M @ x (one projection).

All matmul inputs are bf16 (host-cast); accumulation fp32 in PSUM.

v3 over v2:
- x / wk / wv are host-pretiled into partition-major layouts so every DMA
  is a 2KB-contiguous run per partition, and phase 1 is restructured into
  (stage, si, seg) work units of one o-quarter (2 head-pairs, 256 cols)
  each: only 512KB (wk q0 + x si0) gates the first matmul, quarter-2 units
  for si 0-3 are backfilled during si 4-7, and weights stream just-in-time
  across the SP/Act/Pool queues.
- The per-seg ctx psums accumulate in any si order (sum over s), one PSUM
  bank per seg -- interleaved accumulation groups must NOT share a bank
  (per-bank accumulation state on the PE corrupts results; measured) --
  with the previous unit's ctx matmuls pipelined into the next unit.
- The stage-A AllGather keeps ~27us of stage-B work as skew cover, and its
  bd assembly stays hoisted into the stage-B DVE stream.
- (Evaluated and rejected: fp8 DoubleRow matmuls -- rel err 3.7e-2 exceeds
  the 2e-2 gate; pair-deduplicated M formation via a 1MB AllReduce -- the
  CC fabric moves ~35GB/s so the exchange costs ~30us against 13.7us of PE
  saved and stalls the PE ~20us; PE p-state warmup matmuls -- counter-
  productive under ambient down-clock windows.)
"""

import numpy as np
import ml_dtypes

import jax
from jax.sharding import Mesh, NamedSharding, PartitionSpec

from concourse import bass, bacc, tile, mybir
from concourse.bass2jax import (
    _bass_exec_p,
    install_neuronx_cc_hook,
    partition_id_tensor,
)

from jax.experimental.shard_map import shard_map

F32 = mybir.dt.float32
BF16 = mybir.dt.bfloat16
ACT = mybir.ActivationFunctionType
BF16_NP = ml_dtypes.bfloat16

N_CORES = 8
B, C, S = 4, 1024, 4096
H, DH = 16, 64
S_LOC = S // 2          # per-core sequence columns
NCH = C // 128          # contraction chunks (8)
PAIRS = C // 128        # head pairs = 8 (each pair = 128 channels)
N_SI = S_LOC // 128     # s-chunks per half-loop (16)
N_SB = S_LOC // 512     # out s-blocks (4)
REPLICAS = [[0, 1], [2, 3], [4, 5], [6, 7]]
# phase-1 stages as (first_pair, n_pairs, segments): a 6/2 split so the
# first AllGather covers 6 of 8 head-pairs and the trailing collective
# (2 pairs) hides under the first stage's T1/M^T work. Stage A computes
# its 768 output channels per s-chunk as a 512-wide + 256-wide segment
# to stay within PSUM bank-sized psum tiles.
STAGES6 = [
    (0, 6, [(0, 4), (4, 2)]),
    (6, 2, [(6, 2)]),
]


def build_program(has_kv_bias=False, has_q_bias=False):
    nc = bacc.Bacc(
        "TRN2", target_bir_lowering=False, debug=False, num_devices=N_CORES
    )

    # x is host-pretiled to [128, si, ci, 128] so every per-si DMA is a
    # 2KB-contiguous run per partition (vs 512B with the flat [C, S] layout)
    x_d = nc.dram_tensor("x", [128, N_SI, NCH, 128], BF16, kind="ExternalInput")
    # wk/wv are host-pretiled to [128, o-quarter, ci, 256] so the first
    # (PE-gating) weight tranches are 2KB-contiguous per partition
    wkt_d = nc.dram_tensor("wkt", [128, 4, NCH, 256], BF16, kind="ExternalInput")
    wvt_d = nc.dram_tensor("wvt", [128, 4, NCH, 256], BF16, kind="ExternalInput")
    wqn_d = nc.dram_tensor("wqn", [C, C], BF16, kind="ExternalInput")
    wot_d = nc.dram_tensor("wot", [C, C], BF16, kind="ExternalInput")
    bo_col_d = nc.dram_tensor("bo_col", [128, NCH], F32, kind="ExternalInput")
    bk_d = nc.dram_tensor("bk_row", [1, C], BF16, kind="ExternalInput")
    bv_d = nc.dram_tensor("bv_row", [1, C], BF16, kind="ExternalInput")
    bq_d = nc.dram_tensor("bq_col", [128, NCH], BF16, kind="ExternalInput")
    ones_d = nc.dram_tensor("ones", [1, 128], BF16, kind="ExternalInput")
    # output is emitted bf16 (halves the out-DMA); the host upcasts to f32
    out_d = nc.dram_tensor("out", [C, S_LOC], BF16, kind="ExternalOutput")

    with tile.TileContext(nc) as tc:
        with (
            tc.tile_pool(name="const", bufs=1) as const,
            # kv/tmp stay open for the whole program: closing them mid-run
            # would let phase-2 SBUF tiles reuse their region, serializing
            # the first T1 writes behind the phase-1 drain
            tc.tile_pool(name="kv", bufs=4) as kv,
            tc.tile_pool(name="tmp", bufs=5) as tmp,
            # one program-wide PSUM ring for every matmul output: pool
            # transitions between phases otherwise serialize the next
            # phase's first matmuls behind the previous pool's drain
            tc.tile_pool(name="ps_all", bufs=5, space="PSUM") as ps_all,
            tc.tile_pool(name="ps_ctx", bufs=3, space="PSUM") as ps_ctx,
            tc.tile_pool(name="dram", bufs=1, space="DRAM") as dram,
        ):
            # ---- resident tensors; DMA order = sync queue order ----------
            wk_sb = const.tile([128, 4, NCH, 256], BF16, tag="wk")
            wv_sb = const.tile([128, 4, NCH, 256], BF16, tag="wv")
            x_sb = const.tile([128, N_SI, NCH, 128], BF16, tag="x")
            wk_ap = wkt_d.ap()
            wv_ap = wvt_d.ap()
            x_ap = x_d.ap()

            def wslice(w_sb, ci, ho, w):
                # rhs AP for o-columns [ho, ho+w) of chunk ci in the
                # o-quarter-major layout; ho/w are multiples of 256
                q0 = ho // 256
                return w_sb[:, q0 : q0 + w // 256, ci, :]

            # Two-queue (SP hwdge + Pool swdge) schedule in first-use order.
            # Stage-A's narrow seg (o 512:768) is deferred+backfilled in the
            # unit loop below, so only seg0 weights + x si0 gate the PE.
            # The hwdge queues round-robin their in-flight entries (an
            # entry completes after ~size/(bw/n_coresident)), so keep them
            # nearly empty at the start: the PE-gating pieces ride hwdge
            # alone, and the bulk rides the Pool swdge queue's separate
            # rings in deadline order.
            # sync+scalar share the hwdge rings, and entries round-robin:
            # ONLY the first-matmul gate (x si0 + wk q0, 768KB) rides hwdge
            # so it completes at gate_bytes/bw; everything else rides the
            # Pool swdge queue's separate rings, trigger-ordered by deadline
            nc.sync.dma_start(x_sb[:, 0:1], x_ap[:, 0:1])
            nc.sync.dma_start(wk_sb[:, 0:1, 0:4], wk_ap[:, 0:1, 0:4])
            nc.scalar.dma_start(wk_sb[:, 0:1, 4:8], wk_ap[:, 0:1, 4:8])
            nc.scalar.dma_start(wv_sb[:, 0:1], wv_ap[:, 0:1])
            nc.gpsimd.dma_start(x_sb[:, 1:2], x_ap[:, 1:2])
            nc.gpsimd.dma_start(wk_sb[:, 1:2], wk_ap[:, 1:2])
            nc.gpsimd.dma_start(wv_sb[:, 1:2], wv_ap[:, 1:2])
            bias_tiles = {}
            if has_kv_bias:
                bk_row = const.tile([1, C], BF16, tag="bk")
                nc.sync.dma_start(bk_row[:], bk_d[:])
                bv_row = const.tile([1, C], BF16, tag="bv")
                nc.sync.dma_start(bv_row[:], bv_d[:])
                ones_row = const.tile([1, 128], BF16, tag="ones")
                nc.sync.dma_start(ones_row[:], ones_d[:])
                bias_tiles.update(bk=bk_row, bv=bv_row, ones=ones_row)
            nc.gpsimd.dma_start(x_sb[:, 2:4], x_ap[:, 2:4])
            nc.gpsimd.dma_start(x_sb[:, 4:6], x_ap[:, 4:6])
            nc.gpsimd.dma_start(wk_sb[:, 2:3], wk_ap[:, 2:3])
            nc.gpsimd.dma_start(wv_sb[:, 2:3], wv_ap[:, 2:3])
            nc.gpsimd.dma_start(x_sb[:, 6:8], x_ap[:, 6:8])
            nc.gpsimd.dma_start(x_sb[:, 8:10], x_ap[:, 8:10])
            nc.gpsimd.dma_start(x_sb[:, 10:12], x_ap[:, 10:12])
            nc.gpsimd.dma_start(x_sb[:, 12:14], x_ap[:, 12:14])
            nc.gpsimd.dma_start(x_sb[:, 14:16], x_ap[:, 14:16])
            nc.gpsimd.dma_start(wk_sb[:, 3:4], wk_ap[:, 3:4])
            nc.gpsimd.dma_start(wv_sb[:, 3:4], wv_ap[:, 3:4])

            wqn_sb = const.tile([128, NCH, C], BF16, tag="wqn")
            nc.sync.dma_start(
                wqn_sb[:], wqn_d.ap().rearrange("(n p) c -> p n c", p=128)
            )
            wot_sb = const.tile([128, NCH, C], BF16, tag="wot")
            nc.gpsimd.dma_start(
                wot_sb[:], wot_d.ap().rearrange("(n p) o -> p n o", p=128)
            )
            bo_col = const.tile([128, NCH], F32, tag="bo")
            nc.sync.dma_start(bo_col[:], bo_col_d[:])
            if has_q_bias:
                bq_col = const.tile([128, NCH], BF16, tag="bq")
                nc.sync.dma_start(bq_col[:], bq_d[:])

            # block-diagonal ctx^T lhsT tiles: memset-zeroed up front, diagonal
            # 64x64 blocks overwritten after each AllGather lands.
            bd, raw_ab, ar_in, ar_out = [], [], [], []
            for h, (fp, npr, _segs) in enumerate(STAGES6):
                w = npr * 128
                bd_h = const.tile([128, w], BF16, tag=f"bd{h}", name=f"bd{h}")
                nc.gpsimd.memset(bd_h[:], 0.0)
                bd.append(bd_h)
                raw_ab.append(
                    [
                        const.tile(
                            [128, w], BF16, tag=f"raw{h}{g}", name=f"raw{h}{g}"
                        )
                        for g in range(2)
                    ]
                )
                ar_in.append(
                    dram.tile([128, w], BF16, tag=f"ari{h}", name=f"ari{h}")
                )
                ar_out.append(
                    dram.tile([2, 128, w], BF16, tag=f"aro{h}", name=f"aro{h}")
                )

            # ---------------- Phase 1: k/v projections + context ----------
            # Work units (stage, si, seg), one o-quarter (2 head-pairs, 256
            # cols) each, so the first matmul is gated on just 512KB (wk q0
            # + x si0) and each quarter's weights stream in just-in-time.
            # Quarter 2 is deferred for si 0-3 and backfilled during si 4-7.
            # ctx accumulates per seg in any si order (it's a sum over s);
            # each unit's ctx matmuls are emitted one unit later (software
            # pipelining keeps the PE stream dense while the feature map is
            # in flight). Segs 0+1 share one psum bank (col regions 0:256 /
            # 256:512).
            SEGS = [(0, 2), (2, 2), (4, 2), (6, 2)]  # seg -> (first_pair, n)
            # one PSUM bank per seg: interleaved accumulation groups must
            # not share a bank (per-bank accumulation state on the PE)
            SEG_TILE = [(0, 0), (1, 0), (2, 0), (3, 0)]  # seg -> (bank key, col off)
            units = []
            for si in range(N_SI):
                units.append((0, si, 0))
                units.append((0, si, 1))
                if 4 <= si < 8:
                    units.append((0, si - 4, 2))
                if si >= 4:
                    units.append((0, si, 2))
            for si in range(N_SI):
                units.append((1, si, 3))

            def ctx_dst(sg):
                key, roff = SEG_TILE[sg]
                if key not in ctx_tiles:
                    ctx_tiles[key] = ps_ctx.tile(
                        [128, 512], F32, tag="ctx", name=f"ctx{key}"
                    )  # bank-sized; only [0:256] used
                return ctx_tiles[key], roff

            def emit_stage_gather(st):
                # partial context -> SBUF -> DRAM, AllGather the pair.
                # GPSIMD cannot read PSUM so the evict runs on DVE; the
                # DRAM staging DMA + collective go on the Pool queue, and
                # the gathered results are pulled back on the sync queue.
                fp, npr, _ = STAGES6[st]
                w_st = npr * 128
                ctx_sb = tmp.tile([128, 768], BF16, tag=f"ctxsb{st}")
                if st == 0:
                    for g in range(3):
                        nc.vector.tensor_copy(
                            ctx_sb[:, g * 256 : g * 256 + 256],
                            ctx_tiles[g][:, 0:256],
                        )
                else:
                    nc.vector.tensor_copy(
                        ctx_sb[:, 0:256], ctx_tiles[3][:, 0:256]
                    )
                nc.gpsimd.dma_start(ar_in[st][:], ctx_sb[:, 0:w_st])
                nc.gpsimd.collective_compute(
                    "AllGather",
                    mybir.AluOpType.bypass,
                    replica_groups=REPLICAS,
                    ins=[ar_in[st].opt()],
                    outs=[ar_out[st].opt()],
                )
                nc.sync.dma_start(raw_ab[st][0][:], ar_out[st][0:1, :, :])
                nc.sync.dma_start(raw_ab[st][1][:], ar_out[st][1:2, :, :])

            ctx_tiles = {}
            seg_done = [0, 0, 0, 0]
            prev = None  # (seg, kp_t, vt_t) of the previous unit

            def emit_ctx_of_prev():
                psg, pkp, pvt = prev
                pnp = SEGS[psg][1]
                dst, roff = ctx_dst(psg)
                seg_done[psg] += 1
                for pl in range(pnp):
                    po = roff + pl * 128
                    nc.tensor.matmul(
                        dst[:, po : po + 128],
                        pkp[:, pl * 128 : pl * 128 + 128],
                        pvt[:, pl * 128 : pl * 128 + 128],
                        start=(seg_done[psg] == 1 and pl == 0),
                        stop=(seg_done[psg] == N_SI and pl == pnp - 1),
                        skip_group_check=True,
                    )
                if (
                    psg in (0, 1, 2)
                    and seg_done[0] == N_SI
                    and seg_done[1] == N_SI
                    and seg_done[2] == N_SI
                ):
                    seg_done[psg] += 1  # fire the stage-A gather exactly once
                    emit_stage_gather(0)

            for st, si, sg in units:
                sfp, snp = SEGS[sg]
                ho = sfp * 128
                w = snp * 128
                # k^T chunk [s=128, o=w]
                pk = ps_all.tile([128, 512], F32, tag="pp")
                if has_kv_bias:
                    nc.tensor.matmul(
                        pk[:, 0:w],
                        bias_tiles["ones"][:],
                        bias_tiles["bk"][:, ho : ho + w],
                        start=True,
                        stop=False,
                    )
                for ci in range(NCH):
                    nc.tensor.matmul(
                        pk[:, 0:w],
                        x_sb[:, si, ci, :],
                        wslice(wk_sb, ci, ho, w),
                        start=(ci == 0 and not has_kv_bias),
                        stop=(ci == NCH - 1),
                    )
                # feature map: k' = exp(min(k,0)) + relu(k); min (DVE) and
                # relu (Scalar) read pk in parallel, so the serial chain is
                # 3 deep (min -> exp -> add) instead of 4
                kp_t = kv.tile([128, 512], BF16, tag="kp")
                r_t = tmp.tile([128, 512], F32, tag="r")
                nc.scalar.activation(r_t[:, 0:w], pk[:, 0:w], ACT.Relu)
                m_t = tmp.tile([128, 512], F32, tag="m")
                nc.vector.tensor_scalar(
                    m_t[:, 0:w], pk[:, 0:w], 0.0, None, mybir.AluOpType.min
                )
                nc.scalar.activation(m_t[:, 0:w], m_t[:, 0:w], ACT.Exp)
                nc.vector.tensor_add(kp_t[:, 0:w], m_t[:, 0:w], r_t[:, 0:w])

                # v^T chunk [s=128, o=w]
                pv = ps_all.tile([128, 512], F32, tag="pp")
                if has_kv_bias:
                    nc.tensor.matmul(
                        pv[:, 0:w],
                        bias_tiles["ones"][:],
                        bias_tiles["bv"][:, ho : ho + w],
                        start=True,
                        stop=False,
                    )
                for ci in range(NCH):
                    nc.tensor.matmul(
                        pv[:, 0:w],
                        x_sb[:, si, ci, :],
                        wslice(wv_sb, ci, ho, w),
                        start=(ci == 0 and not has_kv_bias),
                        stop=(ci == NCH - 1),
                    )
                vt_t = kv.tile([128, 512], BF16, tag="vt")
                nc.scalar.activation(vt_t[:, 0:w], pv[:, 0:w], ACT.Copy)

                if prev is not None:
                    emit_ctx_of_prev()
                prev = (sg, kp_t, vt_t)

                # hoist stage A's bd-adds into stage B's DVE stream
                # (its AllGather has landed by now) so the T1 matmuls
                # can start the moment the PE drains phase 1
                if st == 1 and si == 14:
                    fp0, npr0, _s0 = STAGES6[0]
                    raw_a0, raw_b0 = raw_ab[0]
                    for pl in range(npr0):
                        po = pl * 128
                        nc.vector.tensor_add(
                            bd[0][0:64, po : po + 64],
                            raw_a0[0:64, po : po + 64],
                            raw_b0[0:64, po : po + 64],
                        )
                        nc.vector.tensor_add(
                            bd[0][64:128, po + 64 : po + 128],
                            raw_a0[64:128, po + 64 : po + 128],
                            raw_b0[64:128, po + 64 : po + 128],
                        )

            emit_ctx_of_prev()  # flush the final stage-B unit
            emit_stage_gather(1)

            # ---------------- Phase 2: M^T = (wo @ BD @ wq)^T, out = M x --
            with (
                tc.tile_pool(name="p2", bufs=1) as p2,
                tc.tile_pool(name="p2t", bufs=3) as p2t,
            ):
                t1_sb = p2.tile([128, NCH, C], BF16, tag="t1")
                m0_sb = p2.tile([128, NCH, C], BF16, tag="m0")
                mt_sb = p2.tile([128, NCH, C], BF16, tag="mt")
                c1_sb = p2.tile([128, NCH], BF16, tag="c1") if has_q_bias else None

                for stage, (fp, npr, _segs) in enumerate(STAGES6):
                    # sum the gathered pair partials on the diagonal 64x64
                    # blocks only, straight into the zeroed bd tiles
                    # (stage 0's adds were hoisted into the phase-1 stream)
                    raw_a, raw_b = raw_ab[stage]
                    bd_h = bd[stage]
                    if stage > 0:
                        for pl in range(npr):
                            po = pl * 128
                            nc.vector.tensor_add(
                                bd_h[0:64, po : po + 64],
                                raw_a[0:64, po : po + 64],
                                raw_b[0:64, po : po + 64],
                            )
                            nc.vector.tensor_add(
                                bd_h[64:128, po + 64 : po + 128],
                                raw_a[64:128, po + 64 : po + 128],
                                raw_b[64:128, po + 64 : po + 128],
                            )

                    # T1 rows for this stage's pairs: T1[pp] = bd[pp].T @ wq[pp rows]
                    # chalf-major order matches the M loop's consumption
                    # (cc 0-3 read chalf 0), and the PSUM evictions alternate
                    # Scalar/DVE so they keep pace with the M matmul stream
                    for chalf in range(2):
                        co = chalf * 512
                        for pl in range(npr):
                            pp = fp + pl
                            # ps_ctx sits idle in phase 2; using it for the
                            # T1 psums decouples them from the M ring, and
                            # half-split evictions across Scalar+DVE turn
                            # the ring at the T1 matmul pace
                            t1_ps = ps_ctx.tile([128, 512], F32, tag="ctx")
                            nc.tensor.matmul(
                                t1_ps[:],
                                bd_h[:, pl * 128 : pl * 128 + 128],
                                wqn_sb[:, pp, co : co + 512],
                            )
                            nc.scalar.activation(
                                t1_sb[:, pp, co : co + 256],
                                t1_ps[:, 0:256],
                                ACT.Copy,
                            )
                            nc.vector.tensor_copy(
                                t1_sb[:, pp, co + 256 : co + 512],
                                t1_ps[:, 256:512],
                            )

                    # M^T partial over this stage's r-chunks:
                    # stage 0 -> m0_sb (bf16 staging), stage 1 -> += -> mt_sb
                    for cc in range(NCH):
                        for ohalf in range(2):
                            oo = ohalf * 512
                            m_ps = ps_all.tile([128, 512], F32, tag="pp")
                            for rl in range(npr):
                                rr = fp + rl
                                nc.tensor.matmul(
                                    m_ps[:],
                                    t1_sb[:, rr, cc * 128 : cc * 128 + 128],
                                    wot_sb[:, rr, oo : oo + 512],
                                    start=(rl == 0),
                                    stop=(rl == npr - 1),
                                )
                            if stage == 0:
                                nc.vector.tensor_copy(
                                    m0_sb[:, cc, oo : oo + 512], m_ps[:]
                                )
                            else:
                                nc.vector.tensor_add(
                                    mt_sb[:, cc, oo : oo + 512],
                                    m_ps[:],
                                    m0_sb[:, cc, oo : oo + 512],
                                )

                # output bias column: obias = wo @ (BD @ bq) + bo
                if has_q_bias:
                    for stage, (fp, npr, _segs) in enumerate(STAGES6):
                        for pl in range(npr):
                            pp = fp + pl
                            c1_ps = ps_all.tile([128, 512], F32, tag="pp")
                            nc.tensor.matmul(
                                c1_ps[:, 0:1],
                                bd[stage][:, pl * 128 : pl * 128 + 128],
                                bq_col[:, pp : pp + 1],
                            )
                            nc.vector.tensor_copy(c1_sb[:, pp : pp + 1], c1_ps[:, 0:1])
                    obias_col = p2.tile([128, NCH], F32, tag="obias")
                    for oc in range(NCH):
                        ob_ps = ps_all.tile([128, 512], F32, tag="pp")
                        for rc in range(NCH):
                            nc.tensor.matmul(
                                ob_ps[:, 0:1],
                                wot_sb[:, rc, oc * 128 : oc * 128 + 128],
                                c1_sb[:, rc : rc + 1],
                                start=(rc == 0),
                                stop=(rc == NCH - 1),
                            )
                        nc.vector.tensor_add(
                            obias_col[:, oc : oc + 1], ob_ps[:, 0:1], bo_col[:, oc : oc + 1]
                        )
                else:
                    obias_col = bo_col

                # ---- out = M @ x (+obias) ----
                for sb in range(N_SB):
                    ss = sb * 512
                    for oc in range(NCH):
                        po_t = ps_all.tile([128, 512], F32, tag="pp")
                        for ci in range(NCH):
                            nc.tensor.matmul(
                                po_t[:],
                                mt_sb[:, ci, oc * 128 : oc * 128 + 128],
                                x_sb[:, sb * 4 : sb * 4 + 4, ci, :],
                                start=(ci == 0),
                                stop=(ci == NCH - 1),
                            )
                        o_t = p2t.tile([128, 512], BF16, tag="o")
                        if sb == N_SB - 1 and oc == NCH - 1:
                            # split the final tile so its eviction/DMA tail
                            # pipelines across two queues
                            for jo, q in ((0, nc.gpsimd), (256, nc.sync)):
                                nc.scalar.activation(
                                    o_t[:, jo : jo + 256],
                                    po_t[:, jo : jo + 256],
                                    ACT.Identity,
                                    bias=obias_col[:, oc : oc + 1],
                                )
                                q.dma_start(
                                    out_d[
                                        oc * 128 : oc * 128 + 128,
                                        ss + jo : ss + jo + 256,
                                    ],
                                    o_t[:, jo : jo + 256],
                                )
                        else:
                            nc.scalar.activation(
                                o_t[:],
                                po_t[:],
                                ACT.Identity,
                                bias=obias_col[:, oc : oc + 1],
                            )
                            nc.gpsimd.dma_start(
                                out_d[oc * 128 : oc * 128 + 128, ss : ss + 512],
                                o_t[:],
                            )


    nc.compile()
    return nc


# ---------------------------------------------------------------------------
# Host-side runner: mirrors run_bass_via_pjrt's multi-core path but caches the
# jitted callable (no donation) so repeat calls don't retrace.
# ---------------------------------------------------------------------------

_CACHE = {}


def _build_runner(key=(False, False)):
    if key in _CACHE:
        return _CACHE[key]

    install_neuronx_cc_hook()
    nc = build_program(*key)

    partition_name = nc.partition_id_tensor.name if nc.partition_id_tensor else None
    in_names, out_names, out_avals = [], [], []
    for alloc in nc.m.functions[0].allocations:
        if not isinstance(alloc, mybir.MemoryLocationSet):
            continue
        name = alloc.memorylocations[0].name
        if alloc.kind == "ExternalInput":
            if name != partition_name:
                in_names.append(name)
        elif alloc.kind == "ExternalOutput":
            out_names.append(name)
            out_avals.append(
                jax.core.ShapedArray(
                    tuple(alloc.tensor_shape), mybir.dt.np(alloc.dtype)
                )
            )
    n_params = len(in_names)
    all_in_names = list(in_names) + list(out_names)
    if partition_name is not None:
        all_in_names.append(partition_name)

    def _body(*args):
        operands = list(args)
        if partition_name is not None:
            operands.append(partition_id_tensor())
        outs = _bass_exec_p.bind(
            *operands,
            out_avals=tuple(out_avals),
            in_names=tuple(all_in_names),
            out_names=tuple(out_names),
            lowering_input_output_aliases=(),
            sim_require_finite=True,
            sim_require_nnan=True,
            nc=nc,
        )
        return tuple(outs)

    devices = jax.devices()[:N_CORES]
    mesh = Mesh(np.asarray(devices), ("core",))
    n_outs = len(out_names)
    fn = jax.jit(
        shard_map(
            _body,
            mesh=mesh,
            in_specs=(PartitionSpec("core"),) * (n_params + n_outs),
            out_specs=(PartitionSpec("core"),) * n_outs,
            check_rep=False,
        ),
        keep_unused=True,
    )
    sharding = NamedSharding(mesh, PartitionSpec("core"))
    runner = dict(
        fn=fn,
        in_names=in_names,
        out_names=out_names,
        out_avals=out_avals,
        sharding=sharding,
    )
    _CACHE[key] = runner
    return runner


def _pack_inputs(runner, in_maps):
    concat = [
        np.concatenate([np.asarray(m[name]) for m in in_maps], axis=0)
        for name in runner["in_names"]
    ]
    zeros = [
        np.zeros((N_CORES * a.shape[0], *a.shape[1:]), a.dtype)
        for a in runner["out_avals"]
    ]
    sh = runner["sharding"]
    return [jax.device_put(c, sh) for c in concat] + [
        jax.device_put(z, sh) for z in zeros
    ]


def _run(runner, in_maps):
    args = _pack_inputs(runner, in_maps)
    outs = runner["fn"](*args)
    results = []
    for ci in range(N_CORES):
        r = {}
        for i, name in enumerate(runner["out_names"]):
            full = np.asarray(outs[i])
            per = full.reshape(N_CORES, *runner["out_avals"][i].shape)
            r[name] = per[ci]
        results.append(r)
    return results


def make_in_maps(x, wq, bq, wk, bk, wv, bv, wo, bo):
    x = np.asarray(x, np.float32)
    def pret_w(w):
        # w^T [c, o] -> [128p, o-quarter, ci, 256]
        wt = np.asarray(w, np.float32).T
        return np.ascontiguousarray(
            wt.reshape(NCH, 128, 4, 256).transpose(1, 2, 0, 3)
        ).astype(BF16_NP)

    wktb = pret_w(wk)
    wvtb = pret_w(wv)
    wqnb = np.ascontiguousarray(np.asarray(wq, np.float32)).astype(BF16_NP)
    wotb = np.ascontiguousarray(np.asarray(wo, np.float32).T).astype(BF16_NP)
    bqa = np.asarray(bq, np.float32)
    bka = np.asarray(bk, np.float32)
    bva = np.asarray(bv, np.float32)
    boa = np.asarray(bo, np.float32)
    bo_col = np.ascontiguousarray(boa.reshape(NCH, 128).T)
    bq_col = np.ascontiguousarray(bqa.reshape(NCH, 128).T).astype(BF16_NP)
    bk_row = bka.reshape(1, C).astype(BF16_NP)
    bv_row = bva.reshape(1, C).astype(BF16_NP)
    ones = np.ones((1, 128), BF16_NP)
    in_maps = []
    for i in range(N_CORES):
        b, hh = i // 2, i % 2
        xi = x[b, :, hh * S_LOC : (hh + 1) * S_LOC]
        # pretile to [128p, si, ci, 128]: per-si DMAs are 2KB-contiguous
        xi = np.ascontiguousarray(
            xi.reshape(NCH, 128, N_SI, 128).transpose(1, 2, 0, 3)
        ).astype(BF16_NP)
        in_maps.append(
            dict(
                x=xi, wkt=wktb, wvt=wvtb, wqn=wqnb, wot=wotb,
                bo_col=bo_col, bq_col=bq_col, bk_row=bk_row, bv_row=bv_row,
                ones=ones,
            )
        )
    return in_maps


def kernel(x, wq, bq, wk, bk, wv, bv, wo, bo, num_heads):
    assert int(num_heads) == H
    x = np.asarray(x, np.float32)
    assert x.shape == (B, C, S), x.shape

    has_kv_bias = bool(np.any(np.asarray(bk)) or np.any(np.asarray(bv)))
    has_q_bias = bool(np.any(np.asarray(bq)))
    runner = _build_runner((has_kv_bias, has_q_bias))
    in_maps = make_in_maps(x, wq, bq, wk, bk, wv, bv, wo, bo)
    results = _run(runner, in_maps)

    out = np.empty((B, C, S), np.float32)
    for i in range(N_CORES):
        b, hh = i // 2, i % 2
        out[b, :, hh * S_LOC : (hh + 1) * S_LOC] = results[i]["out"]
    return out



# revision 34
# speedup vs baseline: 31.1797x; 22.4857x over previous
"""Linear multi-head attention (ELU+1 feature map) Trainium2 Bass kernel, v3.

Full inputs in, full output out. Sharding: 8 cores = (batch b, seq-half h);
core i handles batch i//2, sequence columns [h*2048, (h+1)*2048).

Math: out = wo @ BD(ctx^T) @ wq @ x + bias terms, where
  ctx[h] = k'[h] @ v[h]^T summed over the full sequence (AllGather over the
  2-core pair), BD = block-diagonal of the per-head ctx^T blocks.
So instead of q-proj / attn / out-proj we form M^T = (BD @ wq)^T-contracted
with wo once per core and apply out = M @ x (one projection).

All matmul inputs are bf16 (host-cast); accumulation fp32 in PSUM.

v3 over v2:
- x / wk / wv are host-pretiled into partition-major layouts so every DMA
  is a 2KB-contiguous run per partition, and phase 1 is restructured into
  (stage, si, seg) work units of one o-quarter (2 head-pairs, 256 cols)
  each: only 512KB (wk q0 + x si0) gates the first matmul, quarter-2 units
  for si 0-3 are backfilled during si 4-7, and weights stream just-in-time
  across the SP/Act/Pool queues.
- The per-seg ctx psums accumulate in any si order (sum over s), one PSUM
  bank per seg -- interleaved accumulation groups must NOT share a bank
  (per-bank accumulation state on the PE corrupts results; measured) --
  with the previous unit's ctx matmuls pipelined into the next unit.
- The stage-A AllGather keeps ~27us of stage-B work as skew cover, and its
  bd assembly stays hoisted into the stage-B DVE stream.
- T1 PSUM evictions are emitted in M's consumption order, half-split across
  Scalar+DVE, with the T1 psums on the phase-2-idle ps_ctx ring.
- The output is emitted bf16 (halved out-DMA; host upcasts to f32), and all
  bulk input DMA rides the Pool swdge queue so nothing round-robin-steals
  hwdge ring bandwidth from the first-matmul gate.
- (Evaluated and rejected: fp8 DoubleRow matmuls -- rel err 3.7e-2 exceeds
  the 2e-2 gate; pair-deduplicated M formation via a 1MB AllReduce -- the
  CC fabric moves ~35GB/s so the exchange costs ~30us against 13.7us of PE
  saved and stalls the PE ~20us; PE p-state warmup matmuls -- counter-
  productive under ambient down-clock windows.)
"""

import numpy as np
import ml_dtypes

import jax
from jax.sharding import Mesh, NamedSharding, PartitionSpec

from concourse import bass, bacc, tile, mybir
from concourse.bass2jax import (
    _bass_exec_p,
    install_neuronx_cc_hook,
    partition_id_tensor,
)

from jax.experimental.shard_map import shard_map

F32 = mybir.dt.float32
BF16 = mybir.dt.bfloat16
ACT = mybir.ActivationFunctionType
BF16_NP = ml_dtypes.bfloat16

N_CORES = 8
B, C, S = 4, 1024, 4096
H, DH = 16, 64
S_LOC = S // 2          # per-core sequence columns
NCH = C // 128          # contraction chunks (8)
PAIRS = C // 128        # head pairs = 8 (each pair = 128 channels)
N_SI = S_LOC // 128     # s-chunks per half-loop (16)
N_SB = S_LOC // 512     # out s-blocks (4)
REPLICAS = [[0, 1], [2, 3], [4, 5], [6, 7]]
# phase-1 stages as (first_pair, n_pairs, segments): a 6/2 split so the
# first AllGather covers 6 of 8 head-pairs and the trailing collective
# (2 pairs) hides under the first stage's T1/M^T work. Stage A computes
# its 768 output channels per s-chunk as a 512-wide + 256-wide segment
# to stay within PSUM bank-sized psum tiles.
STAGES6 = [
    (0, 6, [(0, 4), (4, 2)]),
    (6, 2, [(6, 2)]),
]


def build_program(has_kv_bias=False, has_q_bias=False):
    nc = bacc.Bacc(
        "TRN2", target_bir_lowering=False, debug=False, num_devices=N_CORES
    )

    # x is host-pretiled to [128, si, ci, 128] so every per-si DMA is a
    # 2KB-contiguous run per partition (vs 512B with the flat [C, S] layout)
    x_d = nc.dram_tensor("x", [128, N_SI, NCH, 128], BF16, kind="ExternalInput")
    # wk/wv are host-pretiled to [128, o-quarter, ci, 256] so the first
    # (PE-gating) weight tranches are 2KB-contiguous per partition
    wkt_d = nc.dram_tensor("wkt", [128, 4, NCH, 256], BF16, kind="ExternalInput")
    wvt_d = nc.dram_tensor("wvt", [128, 4, NCH, 256], BF16, kind="ExternalInput")
    wqn_d = nc.dram_tensor("wqn", [C, C], BF16, kind="ExternalInput")
    wot_d = nc.dram_tensor("wot", [C, C], BF16, kind="ExternalInput")
    bo_col_d = nc.dram_tensor("bo_col", [128, NCH], F32, kind="ExternalInput")
    bk_d = nc.dram_tensor("bk_row", [1, C], BF16, kind="ExternalInput")
    bv_d = nc.dram_tensor("bv_row", [1, C], BF16, kind="ExternalInput")
    bq_d = nc.dram_tensor("bq_col", [128, NCH], BF16, kind="ExternalInput")
    ones_d = nc.dram_tensor("ones", [1, 128], BF16, kind="ExternalInput")
    # output is emitted bf16 (halves the out-DMA); the host upcasts to f32
    out_d = nc.dram_tensor("out", [C, S_LOC], BF16, kind="ExternalOutput")

    with tile.TileContext(nc) as tc:
        with (
            tc.tile_pool(name="const", bufs=1) as const,
            # kv/tmp stay open for the whole program: closing them mid-run
            # would let phase-2 SBUF tiles reuse their region, serializing
            # the first T1 writes behind the phase-1 drain
            tc.tile_pool(name="kv", bufs=6) as kv,
            tc.tile_pool(name="tmp", bufs=7) as tmp,
            # one program-wide PSUM ring for every matmul output: pool
            # transitions between phases otherwise serialize the next
            # phase's first matmuls behind the previous pool's drain
            tc.tile_pool(name="ps_all", bufs=5, space="PSUM") as ps_all,
            tc.tile_pool(name="ps_ctx", bufs=3, space="PSUM") as ps_ctx,
            tc.tile_pool(name="dram", bufs=1, space="DRAM") as dram,
        ):
            # ---- resident tensors; DMA order = sync queue order ----------
            wk_sb = const.tile([128, 4, NCH, 256], BF16, tag="wk")
            wv_sb = const.tile([128, 4, NCH, 256], BF16, tag="wv")
            x_sb = const.tile([128, N_SI, NCH, 128], BF16, tag="x")
            wk_ap = wkt_d.ap()
            wv_ap = wvt_d.ap()
            x_ap = x_d.ap()

            def wslice(w_sb, ci, ho, w):
                # rhs AP for o-columns [ho, ho+w) of chunk ci in the
                # o-quarter-major layout; ho/w are multiples of 256
                q0 = ho // 256
                return w_sb[:, q0 : q0 + w // 256, ci, :]

            # Two-queue (SP hwdge + Pool swdge) schedule in first-use order.
            # Stage-A's narrow seg (o 512:768) is deferred+backfilled in the
            # unit loop below, so only seg0 weights + x si0 gate the PE.
            # The hwdge queues round-robin their in-flight entries (an
            # entry completes after ~size/(bw/n_coresident)), so keep them
            # nearly empty at the start: the PE-gating pieces ride hwdge
            # alone, and the bulk rides the Pool swdge queue's separate
            # rings in deadline order.
            # sync+scalar share the hwdge rings, and entries round-robin:
            # ONLY the first-matmul gate (x si0 + wk q0, 768KB) rides hwdge
            # so it completes at gate_bytes/bw; everything else rides the
            # Pool swdge queue's separate rings, trigger-ordered by deadline
            nc.sync.dma_start(x_sb[:, 0:1], x_ap[:, 0:1])
            nc.sync.dma_start(wk_sb[:, 0:1, 0:4], wk_ap[:, 0:1, 0:4])
            nc.scalar.dma_start(wk_sb[:, 0:1, 4:8], wk_ap[:, 0:1, 4:8])
            # equal-sized hwdge entries: under byte-fair RR a 512KB entry
            # finishes at 2x the 256KB pieces' time, so split wv q0
            nc.sync.dma_start(wv_sb[:, 0:1, 0:4], wv_ap[:, 0:1, 0:4])
            nc.scalar.dma_start(wv_sb[:, 0:1, 4:8], wv_ap[:, 0:1, 4:8])
            nc.gpsimd.dma_start(x_sb[:, 1:2], x_ap[:, 1:2])
            nc.gpsimd.dma_start(wk_sb[:, 1:2], wk_ap[:, 1:2])
            nc.gpsimd.dma_start(wv_sb[:, 1:2], wv_ap[:, 1:2])
            bias_tiles = {}
            if has_kv_bias:
                bk_row = const.tile([1, C], BF16, tag="bk")
                nc.sync.dma_start(bk_row[:], bk_d[:])
                bv_row = const.tile([1, C], BF16, tag="bv")
                nc.sync.dma_start(bv_row[:], bv_d[:])
                ones_row = const.tile([1, 128], BF16, tag="ones")
                nc.sync.dma_start(ones_row[:], ones_d[:])
                bias_tiles.update(bk=bk_row, bv=bv_row, ones=ones_row)
            nc.gpsimd.dma_start(x_sb[:, 2:4], x_ap[:, 2:4])
            nc.gpsimd.dma_start(x_sb[:, 4:6], x_ap[:, 4:6])
            nc.gpsimd.dma_start(wk_sb[:, 2:3], wk_ap[:, 2:3])
            nc.gpsimd.dma_start(wv_sb[:, 2:3], wv_ap[:, 2:3])
            nc.gpsimd.dma_start(x_sb[:, 6:8], x_ap[:, 6:8])
            nc.gpsimd.dma_start(x_sb[:, 8:10], x_ap[:, 8:10])
            nc.gpsimd.dma_start(x_sb[:, 10:12], x_ap[:, 10:12])
            nc.gpsimd.dma_start(x_sb[:, 12:14], x_ap[:, 12:14])
            nc.gpsimd.dma_start(x_sb[:, 14:16], x_ap[:, 14:16])
            nc.gpsimd.dma_start(wk_sb[:, 3:4], wk_ap[:, 3:4])
            nc.gpsimd.dma_start(wv_sb[:, 3:4], wv_ap[:, 3:4])

            wqn_sb = const.tile([128, NCH, C], BF16, tag="wqn")
            # Pool, not hwdge: a 2MB hwdge entry would round-robin-steal
            # bandwidth from the first-matmul gate for the whole ramp
            nc.gpsimd.dma_start(
                wqn_sb[:], wqn_d.ap().rearrange("(n p) c -> p n c", p=128)
            )
            wot_sb = const.tile([128, NCH, C], BF16, tag="wot")
            nc.gpsimd.dma_start(
                wot_sb[:], wot_d.ap().rearrange("(n p) o -> p n o", p=128)
            )
            bo_col = const.tile([128, NCH], F32, tag="bo")
            nc.scalar.dma_start(bo_col[:], bo_col_d[:])
            if has_q_bias:
                bq_col = const.tile([128, NCH], BF16, tag="bq")
                nc.sync.dma_start(bq_col[:], bq_d[:])

            # block-diagonal ctx^T lhsT tiles: memset-zeroed up front, diagonal
            # 64x64 blocks overwritten after each AllGather lands.
            bd, raw_ab, ar_in, ar_out = [], [], [], []
            for h, (fp, npr, _segs) in enumerate(STAGES6):
                w = npr * 128
                bd_h = const.tile([128, w], BF16, tag=f"bd{h}", name=f"bd{h}")
                nc.gpsimd.memset(bd_h[:], 0.0)
                bd.append(bd_h)
                raw_ab.append(
                    [
                        const.tile(
                            [128, w], BF16, tag=f"raw{h}{g}", name=f"raw{h}{g}"
                        )
                        for g in range(2)
                    ]
                )
                ar_in.append(
                    dram.tile([128, w], BF16, tag=f"ari{h}", name=f"ari{h}")
                )
                ar_out.append(
                    dram.tile([2, 128, w], BF16, tag=f"aro{h}", name=f"aro{h}")
                )

            # ---------------- Phase 1: k/v projections + context ----------
            # Work units (stage, si, seg), one o-quarter (2 head-pairs, 256
            # cols) each, so the first matmul is gated on just 512KB (wk q0
            # + x si0) and each quarter's weights stream in just-in-time.
            # Quarter 2 is deferred for si 0-3 and backfilled during si 4-7.
            # ctx accumulates per seg in any si order (it's a sum over s);
            # each unit's ctx matmuls are emitted one unit later (software
            # pipelining keeps the PE stream dense while the feature map is
            # in flight). Segs 0+1 share one psum bank (col regions 0:256 /
            # 256:512).
            SEGS = [(0, 2), (2, 2), (4, 2), (6, 2)]  # seg -> (first_pair, n)
            # one PSUM bank per seg: interleaved accumulation groups must
            # not share a bank (per-bank accumulation state on the PE)
            SEG_TILE = [(0, 0), (1, 0), (2, 0), (3, 0)]  # seg -> (bank key, col off)
            units = []
            for si in range(N_SI):
                units.append((0, si, 0))
                units.append((0, si, 1))
                if 4 <= si < 8:
                    units.append((0, si - 4, 2))
                if si >= 4:
                    units.append((0, si, 2))
            for si in range(N_SI):
                units.append((1, si, 3))

            def ctx_dst(sg):
                key, roff = SEG_TILE[sg]
                if key not in ctx_tiles:
                    ctx_tiles[key] = ps_ctx.tile(
                        [128, 512], F32, tag="ctx", name=f"ctx{key}"
                    )  # bank-sized; only [0:256] used
                return ctx_tiles[key], roff

            def emit_stage_gather(st):
                # partial context -> SBUF -> DRAM, AllGather the pair.
                # GPSIMD cannot read PSUM so the evict runs on DVE; the
                # DRAM staging DMA + collective go on the Pool queue, and
                # the gathered results are pulled back on the sync queue.
                fp, npr, _ = STAGES6[st]
                w_st = npr * 128
                ctx_sb = tmp.tile([128, 768], BF16, tag=f"ctxsb{st}")
                if st == 0:
                    for g in range(3):
                        nc.vector.tensor_copy(
                            ctx_sb[:, g * 256 : g * 256 + 256],
                            ctx_tiles[g][:, 0:256],
                        )
                else:
                    nc.vector.tensor_copy(
                        ctx_sb[:, 0:256], ctx_tiles[3][:, 0:256]
                    )
                nc.gpsimd.dma_start(ar_in[st][:], ctx_sb[:, 0:w_st])
                nc.gpsimd.collective_compute(
                    "AllGather",
                    mybir.AluOpType.bypass,
                    replica_groups=REPLICAS,
                    ins=[ar_in[st].opt()],
                    outs=[ar_out[st].opt()],
                )
                nc.sync.dma_start(raw_ab[st][0][:], ar_out[st][0:1, :, :])
                nc.sync.dma_start(raw_ab[st][1][:], ar_out[st][1:2, :, :])

            ctx_tiles = {}
            seg_done = [0, 0, 0, 0]
            prev = None  # (seg, kp_t, vt_t) of the previous unit

            def emit_ctx_of_prev():
                psg, pkp, pvt = prev
                pnp = SEGS[psg][1]
                dst, roff = ctx_dst(psg)
                seg_done[psg] += 1
                for pl in range(pnp):
                    po = roff + pl * 128
                    nc.tensor.matmul(
                        dst[:, po : po + 128],
                        pkp[:, pl * 128 : pl * 128 + 128],
                        pvt[:, pl * 128 : pl * 128 + 128],
                        start=(seg_done[psg] == 1 and pl == 0),
                        stop=(seg_done[psg] == N_SI and pl == pnp - 1),
                        skip_group_check=True,
                    )
                if (
                    psg in (0, 1, 2)
                    and seg_done[0] == N_SI
                    and seg_done[1] == N_SI
                    and seg_done[2] == N_SI
                ):
                    seg_done[psg] += 1  # fire the stage-A gather exactly once
                    emit_stage_gather(0)

            for st, si, sg in units:
                sfp, snp = SEGS[sg]
                ho = sfp * 128
                w = snp * 128
                # k^T chunk [s=128, o=w]
                pk = ps_all.tile([128, 512], F32, tag="pp")
                if has_kv_bias:
                    nc.tensor.matmul(
                        pk[:, 0:w],
                        bias_tiles["ones"][:],
                        bias_tiles["bk"][:, ho : ho + w],
                        start=True,
                        stop=False,
                    )
                for ci in range(NCH):
                    nc.tensor.matmul(
                        pk[:, 0:w],
                        x_sb[:, si, ci, :],
                        wslice(wk_sb, ci, ho, w),
                        start=(ci == 0 and not has_kv_bias),
                        stop=(ci == NCH - 1),
                    )
                # feature map: k' = exp(min(k,0)) + relu(k); min (DVE) and
                # relu (Scalar) read pk in parallel, so the serial chain is
                # 3 deep (min -> exp -> add) instead of 4
                kp_t = kv.tile([128, 512], BF16, tag="kp")
                r_t = tmp.tile([128, 512], F32, tag="r")
                nc.scalar.activation(r_t[:, 0:w], pk[:, 0:w], ACT.Relu)
                m_t = tmp.tile([128, 512], F32, tag="m")
                nc.vector.tensor_scalar(
                    m_t[:, 0:w], pk[:, 0:w], 0.0, None, mybir.AluOpType.min
                )
                nc.scalar.activation(m_t[:, 0:w], m_t[:, 0:w], ACT.Exp)
                nc.vector.tensor_add(kp_t[:, 0:w], m_t[:, 0:w], r_t[:, 0:w])

                # v^T chunk [s=128, o=w]
                pv = ps_all.tile([128, 512], F32, tag="pp")
                if has_kv_bias:
                    nc.tensor.matmul(
                        pv[:, 0:w],
                        bias_tiles["ones"][:],
                        bias_tiles["bv"][:, ho : ho + w],
                        start=True,
                        stop=False,
                    )
                for ci in range(NCH):
                    nc.tensor.matmul(
                        pv[:, 0:w],
                        x_sb[:, si, ci, :],
                        wslice(wv_sb, ci, ho, w),
                        start=(ci == 0 and not has_kv_bias),
                        stop=(ci == NCH - 1),
                    )
                vt_t = kv.tile([128, 512], BF16, tag="vt")
                nc.scalar.activation(vt_t[:, 0:w], pv[:, 0:w], ACT.Copy)

                if prev is not None:
                    emit_ctx_of_prev()
                prev = (sg, kp_t, vt_t)

                # hoist stage A's bd-adds into stage B's DVE stream
                # (its AllGather has landed by now) so the T1 matmuls
                # can start the moment the PE drains phase 1
                if st == 1 and si == 14:
                    fp0, npr0, _s0 = STAGES6[0]
                    raw_a0, raw_b0 = raw_ab[0]
                    for pl in range(npr0):
                        po = pl * 128
                        nc.vector.tensor_add(
                            bd[0][0:64, po : po + 64],
                            raw_a0[0:64, po : po + 64],
                            raw_b0[0:64, po : po + 64],
                        )
                        nc.vector.tensor_add(
                            bd[0][64:128, po + 64 : po + 128],
                            raw_a0[64:128, po + 64 : po + 128],
                            raw_b0[64:128, po + 64 : po + 128],
                        )

            emit_ctx_of_prev()  # flush the final stage-B unit
            emit_stage_gather(1)

            # ---------------- Phase 2: M^T = (wo @ BD @ wq)^T, out = M x --
            with (
                tc.tile_pool(name="p2", bufs=1) as p2,
                tc.tile_pool(name="p2t", bufs=3) as p2t,
            ):
                t1_sb = p2.tile([128, NCH, C], BF16, tag="t1")
                # single M^T buffer: stage 0 copies in, stage 1 adds in-place
                m0_sb = p2.tile([128, NCH, C], BF16, tag="m0")
                mt_sb = m0_sb
                c1_sb = p2.tile([128, NCH], BF16, tag="c1") if has_q_bias else None

                for stage, (fp, npr, _segs) in enumerate(STAGES6):
                    # sum the gathered pair partials on the diagonal 64x64
                    # blocks only, straight into the zeroed bd tiles
                    # (stage 0's adds were hoisted into the phase-1 stream)
                    raw_a, raw_b = raw_ab[stage]
                    bd_h = bd[stage]
                    if stage > 0:
                        for pl in range(npr):
                            po = pl * 128
                            nc.vector.tensor_add(
                                bd_h[0:64, po : po + 64],
                                raw_a[0:64, po : po + 64],
                                raw_b[0:64, po : po + 64],
                            )
                            nc.vector.tensor_add(
                                bd_h[64:128, po + 64 : po + 128],
                                raw_a[64:128, po + 64 : po + 128],
                                raw_b[64:128, po + 64 : po + 128],
                            )

                    # T1 rows for this stage's pairs: T1[pp] = bd[pp].T @ wq[pp rows]
                    # chalf-major order matches the M loop's consumption
                    # (cc 0-3 read chalf 0), and the PSUM evictions alternate
                    # Scalar/DVE so they keep pace with the M matmul stream
                    for chalf in range(2):
                        co = chalf * 512
                        for pl in range(npr):
                            pp = fp + pl
                            # ps_ctx sits idle in phase 2; using it for the
                            # T1 psums decouples them from the M ring, and
                            # half-split evictions across Scalar+DVE turn
                            # the ring at the T1 matmul pace
                            t1_ps = ps_ctx.tile([128, 512], F32, tag="ctx")
                            nc.tensor.matmul(
                                t1_ps[:],
                                bd_h[:, pl * 128 : pl * 128 + 128],
                                wqn_sb[:, pp, co : co + 512],
                            )
                            nc.scalar.activation(
                                t1_sb[:, pp, co : co + 256],
                                t1_ps[:, 0:256],
                                ACT.Copy,
                            )
                            nc.vector.tensor_copy(
                                t1_sb[:, pp, co + 256 : co + 512],
                                t1_ps[:, 256:512],
                            )

                    # M^T partial over this stage's r-chunks:
                    # stage 0 -> m0_sb (bf16 staging), stage 1 -> += -> mt_sb
                    for cc in range(NCH):
                        for ohalf in range(2):
                            oo = ohalf * 512
                            m_ps = ps_all.tile([128, 512], F32, tag="pp")
                            for rl in range(npr):
                                rr = fp + rl
                                nc.tensor.matmul(
                                    m_ps[:],
                                    t1_sb[:, rr, cc * 128 : cc * 128 + 128],
                                    wot_sb[:, rr, oo : oo + 512],
                                    start=(rl == 0),
                                    stop=(rl == npr - 1),
                                )
                            if stage == 0:
                                nc.vector.tensor_copy(
                                    m0_sb[:, cc, oo : oo + 512], m_ps[:]
                                )
                            else:
                                nc.vector.tensor_add(
                                    m0_sb[:, cc, oo : oo + 512],
                                    m_ps[:],
                                    m0_sb[:, cc, oo : oo + 512],
                                )

                # output bias column: obias = wo @ (BD @ bq) + bo
                if has_q_bias:
                    for stage, (fp, npr, _segs) in enumerate(STAGES6):
                        for pl in range(npr):
                            pp = fp + pl
                            c1_ps = ps_all.tile([128, 512], F32, tag="pp")
                            nc.tensor.matmul(
                                c1_ps[:, 0:1],
                                bd[stage][:, pl * 128 : pl * 128 + 128],
                                bq_col[:, pp : pp + 1],
                            )
                            nc.vector.tensor_copy(c1_sb[:, pp : pp + 1], c1_ps[:, 0:1])
                    obias_col = p2.tile([128, NCH], F32, tag="obias")
                    for oc in range(NCH):
                        ob_ps = ps_all.tile([128, 512], F32, tag="pp")
                        for rc in range(NCH):
                            nc.tensor.matmul(
                                ob_ps[:, 0:1],
                                wot_sb[:, rc, oc * 128 : oc * 128 + 128],
                                c1_sb[:, rc : rc + 1],
                                start=(rc == 0),
                                stop=(rc == NCH - 1),
                            )
                        nc.vector.tensor_add(
                            obias_col[:, oc : oc + 1], ob_ps[:, 0:1], bo_col[:, oc : oc + 1]
                        )
                else:
                    obias_col = bo_col

                # ---- out = M @ x (+obias) ----
                for sb in range(N_SB):
                    ss = sb * 512
                    for oc in range(NCH):
                        po_t = ps_all.tile([128, 512], F32, tag="pp")
                        for ci in range(NCH):
                            nc.tensor.matmul(
                                po_t[:],
                                mt_sb[:, ci, oc * 128 : oc * 128 + 128],
                                x_sb[:, sb * 4 : sb * 4 + 4, ci, :],
                                start=(ci == 0),
                                stop=(ci == NCH - 1),
                            )
                        o_t = p2t.tile([128, 512], BF16, tag="o")
                        if sb == N_SB - 1 and oc == NCH - 1:
                            # split the final tile so its eviction/DMA tail
                            # pipelines across two queues
                            for jo, q in ((0, nc.gpsimd), (256, nc.sync)):
                                nc.scalar.activation(
                                    o_t[:, jo : jo + 256],
                                    po_t[:, jo : jo + 256],
                                    ACT.Identity,
                                    bias=obias_col[:, oc : oc + 1],
                                )
                                q.dma_start(
                                    out_d[
                                        oc * 128 : oc * 128 + 128,
                                        ss + jo : ss + jo + 256,
                                    ],
                                    o_t[:, jo : jo + 256],
                                )
                        else:
                            nc.scalar.activation(
                                o_t[:],
                                po_t[:],
                                ACT.Identity,
                                bias=obias_col[:, oc : oc + 1],
                            )
                            nc.gpsimd.dma_start(
                                out_d[oc * 128 : oc * 128 + 128, ss : ss + 512],
                                o_t[:],
                            )


    nc.compile()
    return nc


# ---------------------------------------------------------------------------
# Host-side runner: mirrors run_bass_via_pjrt's multi-core path but caches the
# jitted callable (no donation) so repeat calls don't retrace.
# ---------------------------------------------------------------------------

_CACHE = {}


def _build_runner(key=(False, False)):
    if key in _CACHE:
        return _CACHE[key]

    install_neuronx_cc_hook()
    nc = build_program(*key)

    partition_name = nc.partition_id_tensor.name if nc.partition_id_tensor else None
    in_names, out_names, out_avals = [], [], []
    for alloc in nc.m.functions[0].allocations:
        if not isinstance(alloc, mybir.MemoryLocationSet):
            continue
        name = alloc.memorylocations[0].name
        if alloc.kind == "ExternalInput":
            if name != partition_name:
                in_names.append(name)
        elif alloc.kind == "ExternalOutput":
            out_names.append(name)
            out_avals.append(
                jax.core.ShapedArray(
                    tuple(alloc.tensor_shape), mybir.dt.np(alloc.dtype)
                )
            )
    n_params = len(in_names)
    all_in_names = list(in_names) + list(out_names)
    if partition_name is not None:
        all_in_names.append(partition_name)

    def _body(*args):
        operands = list(args)
        if partition_name is not None:
            operands.append(partition_id_tensor())
        outs = _bass_exec_p.bind(
            *operands,
            out_avals=tuple(out_avals),
            in_names=tuple(all_in_names),
            out_names=tuple(out_names),
            lowering_input_output_aliases=(),
            sim_require_finite=True,
            sim_require_nnan=True,
            nc=nc,
        )
        return tuple(outs)

    devices = jax.devices()[:N_CORES]
    mesh = Mesh(np.asarray(devices), ("core",))
    n_outs = len(out_names)
    fn = jax.jit(
        shard_map(
            _body,
            mesh=mesh,
            in_specs=(PartitionSpec("core"),) * (n_params + n_outs),
            out_specs=(PartitionSpec("core"),) * n_outs,
            check_rep=False,
        ),
        keep_unused=True,
    )
    sharding = NamedSharding(mesh, PartitionSpec("core"))
    runner = dict(
        fn=fn,
        in_names=in_names,
        out_names=out_names,
        out_avals=out_avals,
        sharding=sharding,
    )
    _CACHE[key] = runner
    return runner


def _pack_inputs(runner, in_maps):
    concat = [
        np.concatenate([np.asarray(m[name]) for m in in_maps], axis=0)
        for name in runner["in_names"]
    ]
    zeros = [
        np.zeros((N_CORES * a.shape[0], *a.shape[1:]), a.dtype)
        for a in runner["out_avals"]
    ]
    sh = runner["sharding"]
    return [jax.device_put(c, sh) for c in concat] + [
        jax.device_put(z, sh) for z in zeros
    ]


def _run(runner, in_maps):
    args = _pack_inputs(runner, in_maps)
    outs = runner["fn"](*args)
    results = []
    for ci in range(N_CORES):
        r = {}
        for i, name in enumerate(runner["out_names"]):
            full = np.asarray(outs[i])
            per = full.reshape(N_CORES, *runner["out_avals"][i].shape)
            r[name] = per[ci]
        results.append(r)
    return results


def make_in_maps(x, wq, bq, wk, bk, wv, bv, wo, bo):
    x = np.asarray(x, np.float32)
    def pret_w(w):
        # w^T [c, o] -> [128p, o-quarter, ci, 256]
        wt = np.asarray(w, np.float32).T
        return np.ascontiguousarray(
            wt.reshape(NCH, 128, 4, 256).transpose(1, 2, 0, 3)
        ).astype(BF16_NP)

    wktb = pret_w(wk)
    wvtb = pret_w(wv)
    wqnb = np.ascontiguousarray(np.asarray(wq, np.float32)).astype(BF16_NP)
    wotb = np.ascontiguousarray(np.asarray(wo, np.float32).T).astype(BF16_NP)
    bqa = np.asarray(bq, np.float32)
    bka = np.asarray(bk, np.float32)
    bva = np.asarray(bv, np.float32)
    boa = np.asarray(bo, np.float32)
    bo_col = np.ascontiguousarray(boa.reshape(NCH, 128).T)
    bq_col = np.ascontiguousarray(bqa.reshape(NCH, 128).T).astype(BF16_NP)
    bk_row = bka.reshape(1, C).astype(BF16_NP)
    bv_row = bva.reshape(1, C).astype(BF16_NP)
    ones = np.ones((1, 128), BF16_NP)
    in_maps = []
    for i in range(N_CORES):
        b, hh = i // 2, i % 2
        xi = x[b, :, hh * S_LOC : (hh + 1) * S_LOC]
        # pretile to [128p, si, ci, 128]: per-si DMAs are 2KB-contiguous
        xi = np.ascontiguousarray(
            xi.reshape(NCH, 128, N_SI, 128).transpose(1, 2, 0, 3)
        ).astype(BF16_NP)
        in_maps.append(
            dict(
                x=xi, wkt=wktb, wvt=wvtb, wqn=wqnb, wot=wotb,
                bo_col=bo_col, bq_col=bq_col, bk_row=bk_row, bv_row=bv_row,
                ones=ones,
            )
        )
    return in_maps


def kernel(x, wq, bq, wk, bk, wv, bv, wo, bo, num_heads):
    assert int(num_heads) == H
    x = np.asarray(x, np.float32)
    assert x.shape == (B, C, S), x.shape

    has_kv_bias = bool(np.any(np.asarray(bk)) or np.any(np.asarray(bv)))
    has_q_bias = bool(np.any(np.asarray(bq)))
    runner = _build_runner((has_kv_bias, has_q_bias))
    in_maps = make_in_maps(x, wq, bq, wk, bk, wv, bv, wo, bo)
    results = _run(runner, in_maps)

    out = np.empty((B, C, S), np.float32)
    for i in range(N_CORES):
        b, hh = i // 2, i % 2
        out[b, :, hh * S_LOC : (hh + 1) * S_LOC] = results[i]["out"]
    return out

